# revision 5
# baseline (speedup 1.0000x reference)
"""GCN 3-layer + global_add_pool kernel for Trainium2 (8 NeuronCores, Bass).

Self-contained: kernel(**inputs) -> np.ndarray [G].

Model (reference):
    src/dst = edge_index + self loops; deg over dst; dinv = rsqrt(deg)
    norm_e = dinv[src_e] * dinv[dst_e]
    h = relu(segsum(norm * (h W)[src] by dst) + b)  (x3)
    out = (segsum(h3 by batch) @ Wr + br)[:, 0]

Factorization used: agg_i = dinv_i * segsum_e(g[src_e]),  g = dinv * (h W).

Sharding: graph-block aligned. NC c owns graphs [64c, 64(c+1)) == node block c.
Edges are assigned to the NC owning the *source* node block; each NC computes
partial aggregations for every destination slot; a ReduceScatter(add) then
delivers each NC its own 26624-slot slice, from which it builds the next
layer's gather table (and, after layer 3, pools its 64 graphs).

Slot namespace: dst node n (rank r within its graph-block core k) ->
slot (52*k + r//500)*512 + r%500.  512-wide blocks with 12 trailing trash
slots make every chunk's segment count a fixed 512.
"""
import numpy as np

N = 200000
E = 12800000
G = 512
NCORES = 8          # NeuronCores
QCORES = 8          # Q7 cores per NC (ap_gather lanes)
GB = 64             # graphs per block
BLKS = 52           # 500-dst blocks per graph-block core
SLOT_W = 512        # slots per block (500 real + 12 trash)
SEG_REAL = 500
SLOTS_NC = BLKS * SLOT_W          # 26624 slots per NC
SLOTS_ALL = NCORES * SLOTS_NC     # 212992
TBL_ROWS = SLOTS_NC + 1           # +1 zero row at index 0


def _host_prep(x, edge_index, batch):
    batch = batch.astype(np.int64)
    gb_node = (batch // GB).astype(np.int64)          # graph-block (0..7) per node
    node_start = np.searchsorted(gb_node, np.arange(NCORES))
    n_per = np.searchsorted(gb_node, np.arange(NCORES) + 1) - node_start
    assert n_per.max() <= BLKS * SEG_REAL, n_per.max()

    rank = np.arange(N, dtype=np.int64) - node_start[gb_node]
    slot_local = (rank // SEG_REAL) * SLOT_W + rank % SEG_REAL   # [0, SLOTS_NC)
    slot_global = gb_node * SLOTS_NC + slot_local

    src = edge_index[0].astype(np.int64)
    dst = edge_index[1].astype(np.int64)
    loops = np.arange(N, dtype=np.int64)
    src_all = np.concatenate([src, loops])
    dst_all = np.concatenate([dst, loops])
    deg = np.bincount(dst_all, minlength=N).astype(np.float64)
    dinv = (1.0 / np.sqrt(deg)).astype(np.float32)    # deg >= 1 (self loops)

    e_nc = gb_node[src_all]                            # owning NC (by src)
    e_core = gb_node[dst_all]                          # Q7 core (by dst)
    e_rank = dst_all - node_start[e_core]
    e_block = e_rank // SEG_REAL                       # 0..51
    e_rankb = e_rank % SEG_REAL                        # 0..499
    cell = (e_nc * NCORES + e_core) * BLKS + e_block   # [0, 3328)
    NCELL = NCORES * NCORES * BLKS

    # order edges by (cell, dst)
    key = (cell << 18) | dst_all
    order = np.argsort(key, kind="stable")
    cell_s = cell[order]
    rankb_s = e_rankb[order]
    tbl_idx_s = (slot_local[src_all[order]] + 1).astype(np.int16)  # 1..26624

    cnt = np.bincount(cell, minlength=NCELL)
    cap = int(-(-int(cnt.max()) // 16) * 16)
    cap = max(cap, 16)
    cell_start = np.zeros(NCELL + 1, dtype=np.int64)
    np.cumsum(cnt, out=cell_start[1:])
    pos_in_cell = np.arange(src_all.shape[0], dtype=np.int64) - cell_start[cell_s]

    idx_flat = np.zeros((NCELL, cap), dtype=np.int16)  # pad -> 0 (zero row)
    idx_flat[cell_s, pos_in_cell] = tbl_idx_s

    # ends[cell, r] = # edges in cell with rank < = r  (cumulative counts)
    cnt_rank = np.bincount(cell * SEG_REAL + e_rankb,
                           minlength=NCELL * SEG_REAL).reshape(NCELL, SEG_REAL)
    ends = np.cumsum(cnt_rank, axis=1).astype(np.int16)          # [NCELL, 500]
    ends_full = np.concatenate(
        [ends, np.repeat(ends[:, -1:], SLOT_W - SEG_REAL, axis=1)], axis=1)

    # [nc, core, block, :] -> [nc, block, core, :] -> wrapped [nc, block, 128, :/16]
    def wrap(a, width):
        a = a.reshape(NCORES, NCORES, BLKS, width)
        a = a.transpose(0, 2, 1, 3)                   # [nc, block, core, width]
        a = a.reshape(NCORES, BLKS, NCORES, width // 16, 16)
        a = a.transpose(0, 1, 2, 4, 3)                # [nc, blk, core, 16, w/16]
        return np.ascontiguousarray(a.reshape(NCORES, BLKS, 128, width // 16))

    idx_w = wrap(idx_flat, cap)
    ends_w = wrap(ends_full, SLOT_W)

    # per-NC slot-local helpers
    TP = SLOTS_NC // 128                               # 208 tiles/cols
    blk = slot_local // SLOT_W
    off = slot_local % SLOT_W

    xslots = np.zeros((NCORES, SLOTS_NC, 2), dtype=np.float32)
    dinv_sl = np.zeros((NCORES, SLOTS_NC), dtype=np.float32)
    bpool = np.zeros((NCORES, SLOTS_NC, GB), dtype=np.float32)
    for c in range(NCORES):
        own = np.arange(node_start[c], node_start[c] + n_per[c])
        sl = slot_local[own]
        xslots[c, sl] = x[own]
        dinv_sl[c, sl] = dinv[own]
        bpool[c, sl, batch[own] - GB * c] = 1.0

    # layout A "(p t) f": row r = 208*p + t   (table build / elementwise)
    def layA(a):  # [SLOTS_NC, f] -> [128, TP*f]
        return np.ascontiguousarray(a.reshape(128, -1))

    # layout B "(t p) f": row r = 128*t + p   (pooling tiles)
    def layB(a):
        f = a.shape[-1] if a.ndim > 1 else 1
        return np.ascontiguousarray(
            a.reshape(TP, 128, f).transpose(1, 0, 2).reshape(128, TP * f))

    dinv2 = np.repeat(dinv_sl[:, :, None], 2, axis=2)  # duplicate per feature
    inputs = []
    for c in range(NCORES):
        inputs.append({
            "xslots": layA(xslots[c]),                 # [128, 416] f32
            "dinv_a": layA(dinv2[c]),                  # [128, 416] f32
            "dinv_b": layB(dinv2[c]),                  # [128, 416] f32
            "bpool": layB(bpool[c]),                   # [128, 208*64] f32
            "idx": idx_w[c],                           # [52, 128, cap/16] i16
            "ends": ends_w[c],                         # [52, 128, 32] i16
        })
    return inputs, cap


def _build(cap, W1, b1, W2, b2, W3, b3, Wr, br):
    import concourse.bacc as bacc
    import concourse.mybir as mybir
    from concourse.tile import TileContext

    f32 = mybir.dt.float32
    f16 = mybir.dt.float16
    i16 = mybir.dt.int16
    TP = SLOTS_NC // 128                                  # 208

    nc = bacc.Bacc(num_devices=NCORES)
    t_x = nc.dram_tensor("xslots", [128, TP * 2], f32, kind="ExternalInput")
    t_da = nc.dram_tensor("dinv_a", [128, TP * 2], f32, kind="ExternalInput")
    t_db = nc.dram_tensor("dinv_b", [128, TP * 2], f32, kind="ExternalInput")
    t_bp = nc.dram_tensor("bpool", [128, TP * GB], f32, kind="ExternalInput")
    t_idx = nc.dram_tensor("idx", [BLKS, 128, cap // 16], i16, kind="ExternalInput")
    t_end = nc.dram_tensor("ends", [BLKS, 128, SLOT_W // 16], i16, kind="ExternalInput")
    t_out = nc.dram_tensor("out", [1, GB], f32, kind="ExternalOutput")

    gbuf = [nc.dram_tensor(f"gbuf{l}", [TBL_ROWS * 2], f16) for l in range(3)]
    aggp = [nc.dram_tensor(f"aggp{l}", [SLOTS_ALL, 2], f32) for l in range(3)]
    aggr = [nc.dram_tensor(f"aggr{l}", [SLOTS_NC, 2], f32) for l in range(3)]

    Ws = [np.asarray(W1), np.asarray(W2), np.asarray(W3)]
    bs = [np.asarray(b1), np.asarray(b2), np.asarray(b3)]
    Wr = np.asarray(Wr).reshape(2)
    brv = float(np.asarray(br).reshape(())[()] if np.asarray(br).size == 1 else br[0])

    with TileContext(nc) as tc:
        with tc.tile_pool(name="pers", bufs=1) as pp, \
             tc.tile_pool(name="work", bufs=1) as wp, \
             tc.tile_pool(name="psum", bufs=1, space="PSUM") as psp:

            table = pp.tile([128, TBL_ROWS * 2], f16)      # fp16 pairs, all lanes
            dinv_a = pp.tile([128, TP * 2], f32)
            nc.sync.dma_start(dinv_a[:], t_da[:])

            def build_table(l, src_ap, with_pre):
                """g = dinv * (X @ W_l). src_ap: flat [128, TP*2] f32 layout A."""
                W = Ws[l]
                h = wp.tile([128, TP * 2], f32, tag="tbl_h")
                nc.sync.dma_start(h[:], src_ap)
                hv = h[:].rearrange("p (t f) -> p t f", f=2)
                dv = dinv_a[:].rearrange("p (t f) -> p t f", f=2)
                if with_pre:
                    bprev = bs[l - 1]
                    nc.vector.tensor_mul(h[:], h[:], dinv_a[:])
                    for f in range(2):
                        nc.vector.tensor_scalar_add(hv[:, :, f], hv[:, :, f],
                                                    float(bprev[f]))
                    nc.vector.tensor_scalar_max(h[:], h[:], 0.0)
                g = wp.tile([128, TP * 2], f32, tag="tbl_g")
                gv = g[:].rearrange("p (t f) -> p t f", f=2)
                tmp = wp.tile([128, TP], f32, tag="tbl_t")
                for f in range(2):
                    nc.vector.tensor_scalar_mul(gv[:, :, f], hv[:, :, 0],
                                                float(W[0, f]))
                    nc.vector.tensor_scalar_mul(tmp[:], hv[:, :, 1], float(W[1, f]))
                    nc.vector.tensor_add(gv[:, :, f], gv[:, :, f], tmp[:])
                nc.vector.tensor_mul(g[:], g[:], dinv_a[:])
                gh = wp.tile([128, TP * 2], f16, tag="tbl_f16")
                nc.vector.tensor_copy(gh[:], g[:])
                # zero row 0 then body rows 1..
                z = wp.tile([1, 2], f16, tag="tbl_z")
                nc.vector.memset(z[:], 0)
                nc.sync.dma_start(gbuf[l][0:2], z[:])
                nc.sync.dma_start(
                    gbuf[l][2:].rearrange("(p t) -> p t", p=128), gh[:])
                # broadcast to every partition
                nc.sync.dma_start(
                    table[:],
                    gbuf[l][:].rearrange("(o n) -> o n", o=1)
                    .to_broadcast([128, TBL_ROWS * 2]))

            def layer(l):
                zero1 = wp.tile([128, 1], f32, tag="zero1")
                nc.vector.memset(zero1[:], 0)
                for b in range(BLKS):
                    idx = wp.tile([128, cap // 16], i16, tag="idx")
                    end = wp.tile([128, SLOT_W // 16], i16, tag="end")
                    nc.sync.dma_start(idx[:], t_idx[b])
                    nc.sync.dma_start(end[:], t_end[b])
                    gath = wp.tile([128, cap * 2], f16, tag="gath")
                    nc.gpsimd.ap_gather(gath[:], table[:], idx[:],
                                        channels=128, num_elems=TBL_ROWS,
                                        d=2, num_idxs=cap)
                    scan = wp.tile([128, (cap + 1) * 2], f32, tag="scan")
                    nc.vector.memset(scan[:, 0:2], 0)
                    gv = gath[:].rearrange("p (n f) -> p n f", f=2)
                    sv = scan[:].rearrange("p (n f) -> p n f", f=2)
                    for f in range(2):
                        nc.vector.tensor_tensor_scan(
                            sv[:, 1:, f], gv[:, :, f],
                            zero1[:].to_broadcast([128, cap]),
                            0.0, mybir.AluOpType.add, mybir.AluOpType.add)
                    ext = wp.tile([128, (SLOT_W + 1) * 2], f32, tag="ext")
                    nc.vector.memset(ext[:, 0:2], 0)
                    nc.gpsimd.ap_gather(ext[:, 2:], scan[:], end[:],
                                        channels=128, num_elems=cap + 1,
                                        d=2, num_idxs=SLOT_W)
                    diff = wp.tile([128, SLOT_W * 2], f32, tag="diff")
                    nc.vector.tensor_tensor(
                        out=diff[:], in0=ext[:, 2:], in1=ext[:, 0:SLOT_W * 2],
                        op=mybir.AluOpType.subtract)
                    # lanes 0,16,..,112 -> aggp rows [(52k+b)*512, +512)
                    src8 = diff[:].rearrange("(k s) n -> k s n", s=16)[:, 0, :]
                    dst8 = aggp[l][:].rearrange("(k b s) f -> k b s f",
                                                k=NCORES, b=BLKS)[:, b]
                    nc.sync.dma_start(dst8, src8)
                nc.gpsimd.collective_compute(
                    "ReduceScatter", mybir.AluOpType.add,
                    replica_groups=[list(range(NCORES))],
                    ins=[aggp[l][:]], outs=[aggr[l][:]])

            # ---- run ----
            build_table(0, t_x[:], False)
            layer(0)
            for l in (1, 2):
                build_table(l, aggr[l - 1][:].rearrange("(p t) f -> p (t f)", p=128),
                            True)
                layer(l)

            # ---- h3 + pooling ----
            h3 = wp.tile([128, TP * 2], f32, tag="h3")
            nc.sync.dma_start(
                h3[:].rearrange("p (t f) -> p t f", f=2),
                aggr[2][:].rearrange("(t p) f -> p t f", p=128))
            dinv_b = wp.tile([128, TP * 2], f32, tag="dinvb")
            nc.sync.dma_start(dinv_b[:], t_db[:])
            nc.vector.tensor_mul(h3[:], h3[:], dinv_b[:])
            h3v = h3[:].rearrange("p (t f) -> p t f", f=2)
            for f in range(2):
                nc.vector.tensor_scalar_add(h3v[:, :, f], h3v[:, :, f],
                                            float(bs[2][f]))
            nc.vector.tensor_scalar_max(h3[:], h3[:], 0.0)
            pool = psp.tile([GB, 2], f32, space="PSUM")
            tbpv = t_bp[:].rearrange("p (t m) -> p t m", m=GB)
            CH = 16
            for t0 in range(0, TP, CH):
                bp = wp.tile([128, CH * GB], f32, tag="bpool")
                nw = min(CH, TP - t0)
                nc.sync.dma_start(bp[:, :nw * GB], tbpv[:, t0:t0 + nw, :])
                bpv = bp[:].rearrange("p (t m) -> p t m", m=GB)
                for t in range(t0, t0 + nw):
                    nc.tensor.matmul(pool[:], bpv[:, t - t0, :], h3v[:, t, :],
                                     start=(t == 0), stop=(t == TP - 1))
            res = wp.tile([GB, 1], f32, tag="res")
            tmp2 = wp.tile([GB, 1], f32, tag="res2")
            nc.vector.tensor_scalar_mul(res[:], pool[:, 0:1], float(Wr[0]))
            nc.vector.tensor_scalar_mul(tmp2[:], pool[:, 1:2], float(Wr[1]))
            nc.vector.tensor_add(res[:], res[:], tmp2[:])
            nc.vector.tensor_scalar_add(res[:], res[:], brv)
            nc.sync.dma_start(t_out[:].rearrange("o g -> g o"), res[:])

    nc.finalize()
    return nc


def _make_runner(nc):
    """Persistent jitted 8-core runner (axon PJRT path)."""
    import jax
    import concourse.mybir as mybir
    from concourse import bass2jax
    from jax.sharding import Mesh, PartitionSpec
    from jax.experimental.shard_map import shard_map

    bass2jax.install_neuronx_cc_hook()
    pname = nc.partition_id_tensor.name if nc.partition_id_tensor else None
    in_names, out_names, out_avals, zero_outs = [], [], [], []
    for alloc in nc.m.functions[0].allocations:
        if not isinstance(alloc, mybir.MemoryLocationSet):
            continue
        name = alloc.memorylocations[0].name
        if alloc.kind == "ExternalInput":
            if name != pname:
                in_names.append(name)
        elif alloc.kind == "ExternalOutput":
            shape = tuple(alloc.tensor_shape)
            dt = mybir.dt.np(alloc.dtype)
            out_names.append(name)
            out_avals.append(jax.core.ShapedArray(shape, dt))
            zero_outs.append(np.zeros(shape, dt))
    n_par, n_out = len(in_names), len(out_avals)
    all_in = list(in_names) + list(out_names) + ([pname] if pname else [])
    donate = tuple(range(n_par, n_par + n_out))

    def _body(*args):
        ops = list(args)
        if pname:
            ops.append(bass2jax.partition_id_tensor())
        return tuple(bass2jax._bass_exec_p.bind(
            *ops, out_avals=tuple(out_avals), in_names=tuple(all_in),
            out_names=tuple(out_names), lowering_input_output_aliases=(),
            sim_require_finite=True, sim_require_nnan=True, nc=nc))

    mesh = Mesh(np.asarray(jax.devices()[:NCORES]), ("core",))
    fn = jax.jit(
        shard_map(_body, mesh=mesh,
                  in_specs=(PartitionSpec("core"),) * (n_par + n_out),
                  out_specs=(PartitionSpec("core"),) * n_out,
                  check_rep=False),
        donate_argnums=donate, keep_unused=True)

    from jax.sharding import NamedSharding
    shard = NamedSharding(mesh, PartitionSpec("core"))
    dev_cache = {}

    def run(in_maps):
        key = id(in_maps)
        if key not in dev_cache:
            cat = [np.concatenate([np.asarray(m[n]) for m in in_maps], axis=0)
                   for n in in_names]
            dev_cache[key] = jax.device_put(cat, shard)
        catz = [np.concatenate([z] * NCORES, axis=0) for z in zero_outs]
        outs = fn(*dev_cache[key], *catz)
        jax.block_until_ready(outs)
        o = np.asarray(outs[out_names.index("out")])
        per = o.shape[0] // NCORES
        return np.concatenate(
            [o[c * per:(c + 1) * per][0] for c in range(NCORES)])
    return run


def prepare(x, edge_index, batch, W1, b1, W2, b2, W3, b3, Wr, br):
    x = np.asarray(x, dtype=np.float32)
    inputs, cap = _host_prep(x, np.asarray(edge_index), np.asarray(batch))
    nc = _build(cap, np.asarray(W1, np.float32), np.asarray(b1, np.float32),
                np.asarray(W2, np.float32), np.asarray(b2, np.float32),
                np.asarray(W3, np.float32), np.asarray(b3, np.float32),
                np.asarray(Wr, np.float32), np.asarray(br, np.float32))
    return _make_runner(nc), inputs


def kernel(x, edge_index, batch, W1, b1, W2, b2, W3, b3, Wr, br):
    run, inputs = prepare(x, edge_index, batch, W1, b1, W2, b2, W3, b3, Wr, br)
    return run(inputs).astype(np.float32)


if __name__ == "__main__":
    import reference
    ins = reference.setup_inputs()
    ins = {k: np.asarray(v) for k, v in ins.items()}
    exp = np.asarray(reference.reference(**ins))
    got = kernel(**ins)
    err = np.abs(got - exp).max() / max(np.abs(exp).max(), 1e-6)
    print("rel err:", err)


# revision 6
# speedup vs baseline: 1.1740x; 1.1740x over previous
"""GCN 3-layer + global_add_pool kernel for Trainium2 (8 NeuronCores, Bass).

Self-contained: kernel(**inputs) -> np.ndarray [G].

Model (reference):
    src/dst = edge_index + self loops; deg over dst; dinv = rsqrt(deg)
    norm_e = dinv[src_e] * dinv[dst_e]
    h = relu(segsum(norm * (h W)[src] by dst) + b)  (x3)
    out = (segsum(h3 by batch) @ Wr + br)[:, 0]

Factorization used: agg_i = dinv_i * segsum_e(g[src_e]),  g = dinv * (h W).

Sharding: graph-block aligned. NC c owns graphs [64c, 64(c+1)) == node block c.
Edges are assigned to the NC owning the *source* node block; each NC computes
partial aggregations for every destination slot; a ReduceScatter(add) then
delivers each NC its own 26624-slot slice, from which it builds the next
layer's gather table (and, after layer 3, pools its 64 graphs).

Slot namespace: dst node n (rank r within its graph-block core k) ->
slot (52*k + r//500)*512 + r%500.  512-wide blocks with 12 trailing trash
slots make every chunk's segment count a fixed 512.
"""
import numpy as np

N = 200000
E = 12800000
G = 512
NCORES = 8          # NeuronCores
QCORES = 8          # Q7 cores per NC (ap_gather lanes)
GB = 64             # graphs per block
BLKS = 52           # 500-dst blocks per graph-block core
SLOT_W = 512        # slots per block (500 real + 12 trash)
SEG_REAL = 500
SLOTS_NC = BLKS * SLOT_W          # 26624 slots per NC
SLOTS_ALL = NCORES * SLOTS_NC     # 212992
TBL_ROWS = SLOTS_NC + 1           # +1 zero row at index 0


def _host_prep(x, edge_index, batch):
    batch = batch.astype(np.int64)
    gb_node = (batch // GB).astype(np.int64)          # graph-block (0..7) per node
    node_start = np.searchsorted(gb_node, np.arange(NCORES))
    n_per = np.searchsorted(gb_node, np.arange(NCORES) + 1) - node_start
    assert n_per.max() <= BLKS * SEG_REAL, n_per.max()

    rank = np.arange(N, dtype=np.int64) - node_start[gb_node]
    slot_local = (rank // SEG_REAL) * SLOT_W + rank % SEG_REAL   # [0, SLOTS_NC)
    slot_global = gb_node * SLOTS_NC + slot_local

    src = edge_index[0].astype(np.int64)
    dst = edge_index[1].astype(np.int64)
    loops = np.arange(N, dtype=np.int64)
    src_all = np.concatenate([src, loops])
    dst_all = np.concatenate([dst, loops])
    deg = np.bincount(dst_all, minlength=N).astype(np.float64)
    dinv = (1.0 / np.sqrt(deg)).astype(np.float32)    # deg >= 1 (self loops)

    e_nc = gb_node[src_all]                            # owning NC (by src)
    e_core = gb_node[dst_all]                          # Q7 core (by dst)
    e_rank = dst_all - node_start[e_core]
    e_block = e_rank // SEG_REAL                       # 0..51
    e_rankb = e_rank % SEG_REAL                        # 0..499
    cell = (e_nc * NCORES + e_core) * BLKS + e_block   # [0, 3328)
    NCELL = NCORES * NCORES * BLKS

    # order edges by (cell, dst)
    key = (cell << 18) | dst_all
    order = np.argsort(key, kind="stable")
    cell_s = cell[order]
    rankb_s = e_rankb[order]
    tbl_idx_s = (slot_local[src_all[order]] + 1).astype(np.int16)  # 1..26624

    cnt = np.bincount(cell, minlength=NCELL)
    cap = int(-(-int(cnt.max()) // 16) * 16)
    cap = max(cap, 16)
    cell_start = np.zeros(NCELL + 1, dtype=np.int64)
    np.cumsum(cnt, out=cell_start[1:])
    pos_in_cell = np.arange(src_all.shape[0], dtype=np.int64) - cell_start[cell_s]

    idx_flat = np.zeros((NCELL, cap), dtype=np.int16)  # pad -> 0 (zero row)
    idx_flat[cell_s, pos_in_cell] = tbl_idx_s

    # ends[cell, r] = # edges in cell with rank < = r  (cumulative counts)
    cnt_rank = np.bincount(cell * SEG_REAL + e_rankb,
                           minlength=NCELL * SEG_REAL).reshape(NCELL, SEG_REAL)
    ends = np.cumsum(cnt_rank, axis=1).astype(np.int16)          # [NCELL, 500]
    ends_full = np.concatenate(
        [ends, np.repeat(ends[:, -1:], SLOT_W - SEG_REAL, axis=1)], axis=1)

    # [nc, core, block, :] -> [nc, block, core, :] -> wrapped [nc, block, 128, :/16]
    def wrap(a, width):
        a = a.reshape(NCORES, NCORES, BLKS, width)
        a = a.transpose(0, 2, 1, 3)                   # [nc, block, core, width]
        a = a.reshape(NCORES, BLKS, NCORES, width // 16, 16)
        a = a.transpose(0, 1, 2, 4, 3)                # [nc, blk, core, 16, w/16]
        return np.ascontiguousarray(a.reshape(NCORES, BLKS, 128, width // 16))

    idx_w = wrap(idx_flat, cap)
    ends_w = wrap(ends_full, SLOT_W)

    # per-NC slot-local helpers
    TP = SLOTS_NC // 128                               # 208 tiles/cols
    blk = slot_local // SLOT_W
    off = slot_local % SLOT_W

    xslots = np.zeros((NCORES, SLOTS_NC, 2), dtype=np.float32)
    dinv_sl = np.zeros((NCORES, SLOTS_NC), dtype=np.float32)
    bpool = np.zeros((NCORES, SLOTS_NC, GB), dtype=np.float32)
    for c in range(NCORES):
        own = np.arange(node_start[c], node_start[c] + n_per[c])
        sl = slot_local[own]
        xslots[c, sl] = x[own]
        dinv_sl[c, sl] = dinv[own]
        bpool[c, sl, batch[own] - GB * c] = 1.0

    # layout A "(p t) f": row r = 208*p + t   (table build / elementwise)
    def layA(a):  # [SLOTS_NC, f] -> [128, TP*f]
        return np.ascontiguousarray(a.reshape(128, -1))

    # layout B "(t p) f": row r = 128*t + p   (pooling tiles)
    def layB(a):
        f = a.shape[-1] if a.ndim > 1 else 1
        return np.ascontiguousarray(
            a.reshape(TP, 128, f).transpose(1, 0, 2).reshape(128, TP * f))

    dinv2 = np.repeat(dinv_sl[:, :, None], 2, axis=2)  # duplicate per feature
    inputs = []
    for c in range(NCORES):
        inputs.append({
            "xslots": layA(xslots[c]),                 # [128, 416] f32
            "dinv_a": layA(dinv2[c]),                  # [128, 416] f32
            "dinv_b": layB(dinv2[c]),                  # [128, 416] f32
            "bpool": layB(bpool[c]),                   # [128, 208*64] f32
            "idx": idx_w[c],                           # [52, 128, cap/16] i16
            "ends": ends_w[c],                         # [52, 128, 32] i16
        })
    return inputs, cap


def _build(cap, W1, b1, W2, b2, W3, b3, Wr, br):
    import concourse.bacc as bacc
    import concourse.mybir as mybir
    from concourse.tile import TileContext

    f32 = mybir.dt.float32
    f16 = mybir.dt.float16
    i16 = mybir.dt.int16
    TP = SLOTS_NC // 128                                  # 208

    nc = bacc.Bacc(num_devices=NCORES)
    t_x = nc.dram_tensor("xslots", [128, TP * 2], f32, kind="ExternalInput")
    t_da = nc.dram_tensor("dinv_a", [128, TP * 2], f32, kind="ExternalInput")
    t_db = nc.dram_tensor("dinv_b", [128, TP * 2], f32, kind="ExternalInput")
    t_bp = nc.dram_tensor("bpool", [128, TP * GB], f32, kind="ExternalInput")
    t_idx = nc.dram_tensor("idx", [BLKS, 128, cap // 16], i16, kind="ExternalInput")
    t_end = nc.dram_tensor("ends", [BLKS, 128, SLOT_W // 16], i16, kind="ExternalInput")
    t_out = nc.dram_tensor("out", [1, GB], f32, kind="ExternalOutput")

    gbuf = [nc.dram_tensor(f"gbuf{l}", [TBL_ROWS * 2], f16) for l in range(3)]
    aggp = [nc.dram_tensor(f"aggp{l}", [SLOTS_ALL, 2], f32) for l in range(3)]
    aggr = [nc.dram_tensor(f"aggr{l}", [SLOTS_NC, 2], f32) for l in range(3)]

    Ws = [np.asarray(W1), np.asarray(W2), np.asarray(W3)]
    bs = [np.asarray(b1), np.asarray(b2), np.asarray(b3)]
    Wr = np.asarray(Wr).reshape(2)
    brv = float(np.asarray(br).reshape(())[()] if np.asarray(br).size == 1 else br[0])

    with TileContext(nc) as tc:
        with tc.tile_pool(name="pers", bufs=1) as pp, \
             tc.tile_pool(name="work", bufs=1) as wp, \
             tc.tile_pool(name="psum", bufs=1, space="PSUM") as psp:

            table = pp.tile([128, TBL_ROWS * 2], f16)      # fp16 pairs, all lanes
            dinv_a = pp.tile([128, TP * 2], f32)
            nc.sync.dma_start(dinv_a[:], t_da[:])

            def build_table(l, src_ap, with_pre):
                """g = dinv * (X @ W_l). src_ap: flat [128, TP*2] f32 layout A."""
                W = Ws[l]
                h = wp.tile([128, TP * 2], f32, tag="tbl_h")
                nc.sync.dma_start(h[:], src_ap)
                hv = h[:].rearrange("p (t f) -> p t f", f=2)
                dv = dinv_a[:].rearrange("p (t f) -> p t f", f=2)
                if with_pre:
                    bprev = bs[l - 1]
                    nc.vector.tensor_mul(h[:], h[:], dinv_a[:])
                    for f in range(2):
                        nc.vector.tensor_scalar_add(hv[:, :, f], hv[:, :, f],
                                                    float(bprev[f]))
                    nc.vector.tensor_scalar_max(h[:], h[:], 0.0)
                g = wp.tile([128, TP * 2], f32, tag="tbl_g")
                gv = g[:].rearrange("p (t f) -> p t f", f=2)
                tmp = wp.tile([128, TP], f32, tag="tbl_t")
                for f in range(2):
                    nc.vector.tensor_scalar_mul(gv[:, :, f], hv[:, :, 0],
                                                float(W[0, f]))
                    nc.vector.tensor_scalar_mul(tmp[:], hv[:, :, 1], float(W[1, f]))
                    nc.vector.tensor_add(gv[:, :, f], gv[:, :, f], tmp[:])
                nc.vector.tensor_mul(g[:], g[:], dinv_a[:])
                gh = wp.tile([128, TP * 2], f16, tag="tbl_f16")
                nc.vector.tensor_copy(gh[:], g[:])
                # zero row 0 then body rows 1..
                z = wp.tile([1, 2], f16, tag="tbl_z")
                nc.vector.memset(z[:], 0)
                nc.sync.dma_start(gbuf[l][0:2], z[:])
                nc.sync.dma_start(
                    gbuf[l][2:].rearrange("(p t) -> p t", p=128), gh[:])
                # broadcast to every partition
                nc.sync.dma_start(
                    table[:],
                    gbuf[l][:].rearrange("(o n) -> o n", o=1)
                    .to_broadcast([128, TBL_ROWS * 2]))

            def layer(l):
                zero1 = wp.tile([128, 1], f32, tag="zero1")
                nc.vector.memset(zero1[:], 0)
                for b in range(BLKS):
                    idx = wp.tile([128, cap // 16], i16, tag="idx")
                    end = wp.tile([128, SLOT_W // 16], i16, tag="end")
                    nc.sync.dma_start(idx[:], t_idx[b])
                    nc.sync.dma_start(end[:], t_end[b])
                    gath = wp.tile([128, cap * 2], f16, tag="gath")
                    # gather fp16 PAIRS as single f32 words: d=1 halves the
                    # per-element work if ap_gather cost scales with d
                    nc.gpsimd.ap_gather(gath[:].bitcast(f32), table[:].bitcast(f32),
                                        idx[:], channels=128, num_elems=TBL_ROWS,
                                        d=1, num_idxs=cap)
                    scan = wp.tile([128, (cap + 1) * 2], f32, tag="scan")
                    nc.vector.memset(scan[:, 0:2], 0)
                    gv = gath[:].rearrange("p (n f) -> p n f", f=2)
                    sv = scan[:].rearrange("p (n f) -> p n f", f=2)
                    for f in range(2):
                        nc.vector.tensor_tensor_scan(
                            sv[:, 1:, f], gv[:, :, f],
                            zero1[:].to_broadcast([128, cap]),
                            0.0, mybir.AluOpType.add, mybir.AluOpType.add)
                    ext = wp.tile([128, (SLOT_W + 1) * 2], f32, tag="ext")
                    nc.vector.memset(ext[:, 0:2], 0)
                    nc.gpsimd.ap_gather(ext[:, 2:], scan[:], end[:],
                                        channels=128, num_elems=cap + 1,
                                        d=2, num_idxs=SLOT_W)
                    diff = wp.tile([128, SLOT_W * 2], f32, tag="diff")
                    nc.vector.tensor_tensor(
                        out=diff[:], in0=ext[:, 2:], in1=ext[:, 0:SLOT_W * 2],
                        op=mybir.AluOpType.subtract)
                    # lanes 0,16,..,112 -> aggp rows [(52k+b)*512, +512)
                    src8 = diff[:].rearrange("(k s) n -> k s n", s=16)[:, 0, :]
                    dst8 = aggp[l][:].rearrange("(k b s) f -> k b s f",
                                                k=NCORES, b=BLKS)[:, b]
                    nc.sync.dma_start(dst8, src8)
                nc.gpsimd.collective_compute(
                    "ReduceScatter", mybir.AluOpType.add,
                    replica_groups=[list(range(NCORES))],
                    ins=[aggp[l][:]], outs=[aggr[l][:]])

            # ---- run ----
            build_table(0, t_x[:], False)
            layer(0)
            for l in (1, 2):
                build_table(l, aggr[l - 1][:].rearrange("(p t) f -> p (t f)", p=128),
                            True)
                layer(l)

            # ---- h3 + pooling ----
            h3 = wp.tile([128, TP * 2], f32, tag="h3")
            nc.sync.dma_start(
                h3[:].rearrange("p (t f) -> p t f", f=2),
                aggr[2][:].rearrange("(t p) f -> p t f", p=128))
            dinv_b = wp.tile([128, TP * 2], f32, tag="dinvb")
            nc.sync.dma_start(dinv_b[:], t_db[:])
            nc.vector.tensor_mul(h3[:], h3[:], dinv_b[:])
            h3v = h3[:].rearrange("p (t f) -> p t f", f=2)
            for f in range(2):
                nc.vector.tensor_scalar_add(h3v[:, :, f], h3v[:, :, f],
                                            float(bs[2][f]))
            nc.vector.tensor_scalar_max(h3[:], h3[:], 0.0)
            pool = psp.tile([GB, 2], f32, space="PSUM")
            tbpv = t_bp[:].rearrange("p (t m) -> p t m", m=GB)
            CH = 16
            for t0 in range(0, TP, CH):
                bp = wp.tile([128, CH * GB], f32, tag="bpool")
                nw = min(CH, TP - t0)
                nc.sync.dma_start(bp[:, :nw * GB], tbpv[:, t0:t0 + nw, :])
                bpv = bp[:].rearrange("p (t m) -> p t m", m=GB)
                for t in range(t0, t0 + nw):
                    nc.tensor.matmul(pool[:], bpv[:, t - t0, :], h3v[:, t, :],
                                     start=(t == 0), stop=(t == TP - 1))
            res = wp.tile([GB, 1], f32, tag="res")
            tmp2 = wp.tile([GB, 1], f32, tag="res2")
            nc.vector.tensor_scalar_mul(res[:], pool[:, 0:1], float(Wr[0]))
            nc.vector.tensor_scalar_mul(tmp2[:], pool[:, 1:2], float(Wr[1]))
            nc.vector.tensor_add(res[:], res[:], tmp2[:])
            nc.vector.tensor_scalar_add(res[:], res[:], brv)
            nc.sync.dma_start(t_out[:].rearrange("o g -> g o"), res[:])

    nc.finalize()
    return nc


def _make_runner(nc):
    """Persistent jitted 8-core runner (axon PJRT path)."""
    import jax
    import concourse.mybir as mybir
    from concourse import bass2jax
    from jax.sharding import Mesh, PartitionSpec
    from jax.experimental.shard_map import shard_map

    bass2jax.install_neuronx_cc_hook()
    pname = nc.partition_id_tensor.name if nc.partition_id_tensor else None
    in_names, out_names, out_avals, zero_outs = [], [], [], []
    for alloc in nc.m.functions[0].allocations:
        if not isinstance(alloc, mybir.MemoryLocationSet):
            continue
        name = alloc.memorylocations[0].name
        if alloc.kind == "ExternalInput":
            if name != pname:
                in_names.append(name)
        elif alloc.kind == "ExternalOutput":
            shape = tuple(alloc.tensor_shape)
            dt = mybir.dt.np(alloc.dtype)
            out_names.append(name)
            out_avals.append(jax.core.ShapedArray(shape, dt))
            zero_outs.append(np.zeros(shape, dt))
    n_par, n_out = len(in_names), len(out_avals)
    all_in = list(in_names) + list(out_names) + ([pname] if pname else [])
    donate = tuple(range(n_par, n_par + n_out))

    def _body(*args):
        ops = list(args)
        if pname:
            ops.append(bass2jax.partition_id_tensor())
        return tuple(bass2jax._bass_exec_p.bind(
            *ops, out_avals=tuple(out_avals), in_names=tuple(all_in),
            out_names=tuple(out_names), lowering_input_output_aliases=(),
            sim_require_finite=True, sim_require_nnan=True, nc=nc))

    mesh = Mesh(np.asarray(jax.devices()[:NCORES]), ("core",))
    fn = jax.jit(
        shard_map(_body, mesh=mesh,
                  in_specs=(PartitionSpec("core"),) * (n_par + n_out),
                  out_specs=(PartitionSpec("core"),) * n_out,
                  check_rep=False),
        donate_argnums=donate, keep_unused=True)

    from jax.sharding import NamedSharding
    shard = NamedSharding(mesh, PartitionSpec("core"))
    dev_cache = {}

    def run(in_maps):
        key = id(in_maps)
        if key not in dev_cache:
            cat = [np.concatenate([np.asarray(m[n]) for m in in_maps], axis=0)
                   for n in in_names]
            dev_cache[key] = jax.device_put(cat, shard)
        catz = [np.concatenate([z] * NCORES, axis=0) for z in zero_outs]
        outs = fn(*dev_cache[key], *catz)
        jax.block_until_ready(outs)
        o = np.asarray(outs[out_names.index("out")])
        per = o.shape[0] // NCORES
        return np.concatenate(
            [o[c * per:(c + 1) * per][0] for c in range(NCORES)])
    return run


def prepare(x, edge_index, batch, W1, b1, W2, b2, W3, b3, Wr, br):
    x = np.asarray(x, dtype=np.float32)
    inputs, cap = _host_prep(x, np.asarray(edge_index), np.asarray(batch))
    nc = _build(cap, np.asarray(W1, np.float32), np.asarray(b1, np.float32),
                np.asarray(W2, np.float32), np.asarray(b2, np.float32),
                np.asarray(W3, np.float32), np.asarray(b3, np.float32),
                np.asarray(Wr, np.float32), np.asarray(br, np.float32))
    return _make_runner(nc), inputs


def kernel(x, edge_index, batch, W1, b1, W2, b2, W3, b3, Wr, br):
    run, inputs = prepare(x, edge_index, batch, W1, b1, W2, b2, W3, b3, Wr, br)
    return run(inputs).astype(np.float32)


if __name__ == "__main__":
    import reference
    ins = reference.setup_inputs()
    ins = {k: np.asarray(v) for k, v in ins.items()}
    exp = np.asarray(reference.reference(**ins))
    got = kernel(**ins)
    err = np.abs(got - exp).max() / max(np.abs(exp).max(), 1e-6)
    print("rel err:", err)
